# revision 3
# baseline (speedup 1.0000x reference)
"""Gated GCN layer (DDI message passing) on 8 Trainium2 NeuronCores.

Strategy (data-parallel over batch B=256 -> 32 sentences/core):
  - Host: per-sentence edge list -> dense [200,200] adjacency matrix M
    (M[src,dst] += data). Aggregations become dense matmuls:
      in_t  = M^T @ (X W_in + b)   ;  out_t = M @ (X W_out + b)
  - X^T is pre-transposed on host and augmented with a ones-row so the
    bias rides inside the matmul (K = 361).
  - One fused weight matrix wt [361, 3*364]: groups (W_in|w_gin),
    (W_out|w_gout), (W_loop|w_gloop) each 361 cols padded to 364; bias row.
  - Device per sentence: Z = Xaug^T.T @ wt (3 K-chunks accumulated in
    PSUM), aggregate with M/M^T as stationary operands (gate column rides
    along), then gating epilogue (one batched sigmoid + STT chain).
    Aggregation runs one sentence behind the main matmul so PSUM slot
    reuse never stalls the PE.
  - Everything bf16 on device (fp32 accumulation in PSUM).

SBUF z-tile layout per (sentence, row-tile): [128, 5*364] bf16 groups:
  g0 in-Z | g1 out-Z | g2 loop-Z | g3 agg-in | g4 agg-out
  each group: 360 feats + gate col at 360 (gates of g2..g4 -> one sigmoid).
"""
import sys

if "/opt/trn_rl_repo" not in sys.path:
    sys.path.insert(0, "/opt/trn_rl_repo")

from contextlib import ExitStack

import ml_dtypes
import numpy as np

B, NN, EE, DIN, DOUT = 256, 200, 400, 360, 360
NCORES = 8
SPC = B // NCORES          # 32 sentences per core
ROWS = SPC * NN            # 6400 rows per core
KA = DIN + 1               # 361: augmented contraction (ones row for bias)
GP = 364                   # group stride in SBUF (4B aligned for bf16)
PSG = 512                  # group stride in PSUM (fp32 bank)
KCH = [(0, 121), (121, 241), (241, 361)]   # K chunks <= 128
NBF16 = np.dtype(ml_dtypes.bfloat16)

_compiled = None


def _build():
    import concourse.bacc as bacc
    import concourse.mybir as mybir
    from concourse.tile import TileContext

    BF16 = mybir.dt.bfloat16
    F32 = mybir.dt.float32
    AF = mybir.ActivationFunctionType
    OP = mybir.AluOpType

    nc = bacc.Bacc(None, target_bir_lowering=False)
    xt_d = nc.dram_tensor("xt", [KA, ROWS], BF16, kind="ExternalInput")
    wt_d = nc.dram_tensor("wt", [KA, 3 * GP], BF16, kind="ExternalInput")
    mf_d = nc.dram_tensor("mf", [SPC, 128, 2 * NN], BF16, kind="ExternalInput")
    mb_d = nc.dram_tensor("mb", [SPC, 128, 2 * NN], BF16, kind="ExternalInput")
    out_d = nc.dram_tensor("out", [SPC, 128, 2 * DOUT], BF16, kind="ExternalOutput")

    with TileContext(nc) as tc, ExitStack() as ctx:
        cpool = ctx.enter_context(tc.tile_pool(name="cpool", bufs=1))
        mpool = ctx.enter_context(tc.tile_pool(name="mpool", bufs=3))
        spool = ctx.enter_context(tc.tile_pool(name="spool", bufs=4))
        tpool = ctx.enter_context(tc.tile_pool(name="tpool", bufs=4))
        zpool = ctx.enter_context(tc.tile_pool(name="zpool", bufs=2, space="PSUM"))
        apool = ctx.enter_context(tc.tile_pool(name="apool", bufs=1, space="PSUM"))

        # ---- resident inputs: X^T (3 K-tiles) and weights ----
        xt_tiles = []
        for kc, (k0, k1) in enumerate(KCH):
            t = cpool.tile([k1 - k0, ROWS], BF16, name=f"xt{kc}")
            for j in range(0, ROWS, 1600):       # chunked so sentence 0 starts early
                nc.sync.dma_start(out=t[:, j:j + 1600], in_=xt_d[k0:k1, j:j + 1600])
            xt_tiles.append(t)
        wt_tiles = []
        for kc, (k0, k1) in enumerate(KCH):
            t = cpool.tile([k1 - k0, 3 * GP], BF16, name=f"wt{kc}")
            nc.sync.dma_start(out=t, in_=wt_d[k0:k1, :])
            wt_tiles.append(t)

        state = {}   # per in-flight sentence: (mf_t, mb_t, [zs0, zs1])

        def emit_main(s):
            mf_t = mpool.tile([128, 2 * NN], BF16, tag="mf", name=f"mf{s}")
            mb_t = mpool.tile([128, 2 * NN], BF16, tag="mb", name=f"mb{s}")
            nc.gpsimd.dma_start(out=mf_t, in_=mf_d[s])
            nc.gpsimd.dma_start(out=mb_t, in_=mb_d[s])
            z_sb = []
            for mt, rows in ((0, 128), (1, 72)):
                c0 = s * NN + mt * 128
                zp = zpool.tile([128, 3 * PSG], F32, tag="z", name=f"zp{s}_{mt}")
                for g in range(3):
                    for kc in range(3):
                        nc.tensor.matmul(
                            zp[0:rows, g * PSG:g * PSG + 361],
                            lhsT=xt_tiles[kc][:, c0:c0 + rows],
                            rhs=wt_tiles[kc][:, g * GP:g * GP + 361],
                            start=(kc == 0), stop=(kc == 2),
                        )
                zs = spool.tile([128, 5 * GP], BF16, tag=f"zs{mt}", name=f"zs{s}_{mt}")
                src = zp[0:rows, :].rearrange("p (g c) -> p g c", g=3)[:, :, 0:GP]
                dst = zs[0:rows, 0:3 * GP].rearrange("p (g c) -> p g c", g=3)
                if mt == 0:
                    nc.vector.tensor_copy(dst, src)
                else:
                    nc.scalar.copy(dst, src)
                z_sb.append(zs)
            state[s] = (mf_t, mb_t, z_sb)

        def emit_agg(s):
            mf_t, mb_t, z_sb = state.pop(s)
            ot = tpool.tile([128, 2 * DOUT], BF16, tag="ot", name=f"ot{s}")
            for mt, rows in ((0, 128), (1, 72)):
                d0 = mt * 128
                ap_ = apool.tile([128, 2 * PSG], F32, tag="agg", name=f"ap{s}_{mt}")
                nc.tensor.matmul(ap_[0:rows, 0:361], lhsT=mf_t[0:128, d0:d0 + rows],
                                 rhs=z_sb[0][0:128, 0:361], start=True, stop=False)
                nc.tensor.matmul(ap_[0:rows, 0:361], lhsT=mf_t[0:72, NN + d0:NN + d0 + rows],
                                 rhs=z_sb[1][0:72, 0:361], start=False, stop=True)
                nc.tensor.matmul(ap_[0:rows, PSG:PSG + 361], lhsT=mb_t[0:128, d0:d0 + rows],
                                 rhs=z_sb[0][0:128, GP:GP + 361], start=True, stop=False)
                nc.tensor.matmul(ap_[0:rows, PSG:PSG + 361], lhsT=mb_t[0:72, NN + d0:NN + d0 + rows],
                                 rhs=z_sb[1][0:72, GP:GP + 361], start=False, stop=True)

                zs = z_sb[mt]
                src = ap_[0:rows, :].rearrange("p (g c) -> p g c", g=2)[:, :, 0:GP]
                dst = zs[0:rows, 3 * GP:5 * GP].rearrange("p (g c) -> p g c", g=2)
                if mt == 0:
                    nc.scalar.copy(dst, src)
                else:
                    nc.vector.tensor_copy(dst, src)

                # ---- gating epilogue ----
                # one sigmoid over the 3 gate cols (loop, agg-in, agg-out)
                sgt = tpool.tile([128, 3], F32, tag="sgt", name=f"sg{s}_{mt}")
                gates = zs[0:rows, :].rearrange("p (g c) -> p g c", c=GP)[:, 2:5, 360]
                nc.scalar.activation(sgt[0:rows], gates, AF.Sigmoid)
                sg_l, sg_i, sg_o = sgt[:, 0:1], sgt[:, 1:2], sgt[:, 2:3]

                t1 = tpool.tile([128, DOUT], BF16, tag="t1", name=f"t1{s}_{mt}")
                t2 = tpool.tile([128, DOUT], BF16, tag="t2", name=f"t2{s}_{mt}")
                t3 = tpool.tile([128, DOUT], BF16, tag="t3", name=f"t3{s}_{mt}")
                nc.vector.tensor_scalar_mul(
                    t1[0:rows], zs[0:rows, 4 * GP:4 * GP + 360], sg_o[0:rows])
                nc.vector.scalar_tensor_tensor(
                    out=t2[0:rows], in0=zs[0:rows, 3 * GP:3 * GP + 360], scalar=sg_i[0:rows],
                    in1=t1[0:rows], op0=OP.mult, op1=OP.add)
                nc.vector.scalar_tensor_tensor(
                    out=t3[0:rows], in0=zs[0:rows, 2 * GP:2 * GP + 360], scalar=sg_l[0:rows],
                    in1=t2[0:rows], op0=OP.mult, op1=OP.add)
                nc.vector.tensor_scalar_max(
                    ot[0:rows, mt * DOUT:(mt + 1) * DOUT], t3[0:rows], 0.0)
            nc.gpsimd.dma_start(out=out_d[s], in_=ot)

        # software pipeline: aggregation runs one sentence behind main
        for s in range(SPC):
            emit_main(s)
            if s > 0:
                emit_agg(s - 1)
        emit_agg(SPC - 1)

    nc.compile()
    return nc


def _get_compiled():
    global _compiled
    if _compiled is None:
        _compiled = _build()
    return _compiled


def kernel(gcn_in, adj_ind, adj_data, w_in, b_in, w_out, b_out, w_loop,
           w_gin, b_gin, w_gout, b_gout, w_gloop):
    from concourse.bass_utils import run_bass_kernel_spmd

    x = np.asarray(gcn_in, np.float32)           # [B, N, DIN]
    idx = np.asarray(adj_ind)[0]                 # [B, E, 2] int
    dat = np.asarray(adj_data, np.float32)[0]    # [B, E]

    # fused weight matrix with bias row
    wt = np.zeros((KA, 3 * GP), np.float32)
    for g, (w, gw, bias, gb) in enumerate([
        (w_in, w_gin, b_in, b_gin),
        (w_out, w_gout, b_out, b_gout),
        (w_loop, w_gloop, None, None),
    ]):
        wt[0:DIN, g * GP:g * GP + DOUT] = np.asarray(w, np.float32)
        wt[0:DIN, g * GP + DOUT] = np.asarray(gw, np.float32)[:, 0]
        if bias is not None:
            wt[DIN, g * GP:g * GP + DOUT] = np.asarray(bias, np.float32)[0]
            wt[DIN, g * GP + DOUT] = np.asarray(gb, np.float32)[0]
    wt = wt.astype(NBF16)

    # dense per-sentence adjacency matrices
    M = np.zeros((B, NN, NN), np.float32)
    bi = np.broadcast_to(np.arange(B)[:, None], idx.shape[:2])
    np.add.at(M, (bi, idx[:, :, 0].astype(np.int64), idx[:, :, 1].astype(np.int64)), dat)

    def chunked(mm):      # [SPC,200,200] -> [SPC,128,400]: two 128-row chunks side by side
        out = np.zeros((SPC, 128, 2 * NN), np.float32)
        out[:, :, 0:NN] = mm[:, 0:128, :]
        out[:, 0:72, NN:2 * NN] = mm[:, 128:200, :]
        return out.astype(NBF16)

    nc = _get_compiled()
    in_maps = []
    for c in range(NCORES):
        xc = x[c * SPC:(c + 1) * SPC].reshape(ROWS, DIN)
        xt = np.empty((KA, ROWS), np.float32)
        xt[0:DIN] = xc.T
        xt[DIN] = 1.0
        mc = M[c * SPC:(c + 1) * SPC]
        in_maps.append({
            "xt": np.ascontiguousarray(xt).astype(NBF16),
            "wt": wt,
            "mf": chunked(mc),
            "mb": chunked(np.ascontiguousarray(mc.transpose(0, 2, 1))),
        })

    res = run_bass_kernel_spmd(nc, in_maps, core_ids=list(range(NCORES)))
    kernel.last_results = res
    out = np.empty((B, NN, DOUT), np.float32)
    for c in range(NCORES):
        oc = res.results[c]["out"].astype(np.float32)   # [SPC,128,720]
        oc_s = out[c * SPC:(c + 1) * SPC]               # [SPC,200,360]
        oc_s[:, 0:128, :] = oc[:, :, 0:DOUT]
        oc_s[:, 128:200, :] = oc[:, 0:72, DOUT:2 * DOUT]
    return out


# revision 5
# speedup vs baseline: 1.2489x; 1.2489x over previous
"""Gated GCN layer (DDI message passing) on 8 Trainium2 NeuronCores.

Strategy (data-parallel over batch B=256 -> 32 sentences/core):
  - Host: per-sentence edge list -> dense [200,200] adjacency matrix M
    (M[src,dst] += data). Aggregations become dense matmuls:
      in_t  = M^T @ (X W_in + b)   ;  out_t = M @ (X W_out + b)
  - X^T is pre-transposed on host and augmented with a ones-row so the
    bias rides inside the matmul (K = 361).
  - One fused weight matrix wt [361, 3*364]: groups (W_in|w_gin),
    (W_out|w_gout), (W_loop|w_gloop) each 361 cols padded to 364; bias row.
  - Device per sentence: Z = Xaug^T.T @ wt (3 K-chunks accumulated in
    PSUM), aggregate with M/M^T as stationary operands (gate column rides
    along), then gating epilogue (one batched sigmoid + STT chain).
    Aggregation runs one sentence behind the main matmul so PSUM slot
    reuse never stalls the PE.
  - Everything bf16 on device (fp32 accumulation in PSUM).

SBUF z-tile layout per (sentence, row-tile): [128, 5*364] bf16 groups:
  g0 in-Z | g1 out-Z | g2 loop-Z | g3 agg-in | g4 agg-out
  each group: 360 feats + gate col at 360 (gates of g2..g4 -> one sigmoid).
"""
import sys

if "/opt/trn_rl_repo" not in sys.path:
    sys.path.insert(0, "/opt/trn_rl_repo")

from contextlib import ExitStack

import ml_dtypes
import numpy as np

B, NN, EE, DIN, DOUT = 256, 200, 400, 360, 360
NCORES = 8
SPC = B // NCORES          # 32 sentences per core
ROWS = SPC * NN            # 6400 rows per core
KA = DIN + 1               # 361: augmented contraction (ones row for bias)
GP = 364                   # group stride in SBUF (4B aligned for bf16)
PSG = 512                  # group stride in PSUM (fp32 bank)
KCH = [(0, 121), (121, 241), (241, 361)]   # K chunks <= 128
NBF16 = np.dtype(ml_dtypes.bfloat16)

_compiled = None


def _build():
    import concourse.bacc as bacc
    import concourse.mybir as mybir
    from concourse.tile import TileContext

    BF16 = mybir.dt.bfloat16
    F32 = mybir.dt.float32
    AF = mybir.ActivationFunctionType
    OP = mybir.AluOpType

    nc = bacc.Bacc(None, target_bir_lowering=False)
    xt_d = nc.dram_tensor("xt", [KA, ROWS], BF16, kind="ExternalInput")
    wt_d = nc.dram_tensor("wt", [KA, 3 * GP], BF16, kind="ExternalInput")
    mf_d = nc.dram_tensor("mf", [SPC, 128, 2 * NN], BF16, kind="ExternalInput")
    mb_d = nc.dram_tensor("mb", [SPC, 128, 2 * NN], BF16, kind="ExternalInput")
    out_d = nc.dram_tensor("out", [SPC, 128, 2 * DOUT], BF16, kind="ExternalOutput")

    with TileContext(nc) as tc, ExitStack() as ctx:
        cpool = ctx.enter_context(tc.tile_pool(name="cpool", bufs=1))
        mpool = ctx.enter_context(tc.tile_pool(name="mpool", bufs=3))
        spool = ctx.enter_context(tc.tile_pool(name="spool", bufs=4))
        tpool = ctx.enter_context(tc.tile_pool(name="tpool", bufs=4))
        zpool = ctx.enter_context(tc.tile_pool(name="zpool", bufs=2, space="PSUM"))
        apool = ctx.enter_context(tc.tile_pool(name="apool", bufs=1, space="PSUM"))

        # ---- resident inputs: weights first (tiny), then X^T with the
        # first sentences' columns landing first ----
        wt_tiles = []
        for kc, (k0, k1) in enumerate(KCH):
            t = cpool.tile([k1 - k0, 3 * GP], BF16, name=f"wt{kc}")
            nc.sync.dma_start(out=t, in_=wt_d[k0:k1, :])
            wt_tiles.append(t)
        xt_tiles = [cpool.tile([k1 - k0, ROWS], BF16, name=f"xt{kc}")
                    for kc, (k0, k1) in enumerate(KCH)]
        for j in list(range(0, 3200, 800)) + list(range(3200, ROWS, 1600)):
            w = 800 if j < 3200 else 1600
            for kc, (k0, k1) in enumerate(KCH):
                nc.sync.dma_start(out=xt_tiles[kc][:, j:j + w], in_=xt_d[k0:k1, j:j + w])

        state = {}   # per in-flight sentence: (mf_t, mb_t, [zs0, zs1])

        def emit_main(s):
            mf_t = mpool.tile([128, 2 * NN], BF16, tag="mf", name=f"mf{s}")
            mb_t = mpool.tile([128, 2 * NN], BF16, tag="mb", name=f"mb{s}")
            nc.gpsimd.dma_start(out=mf_t, in_=mf_d[s])
            nc.gpsimd.dma_start(out=mb_t, in_=mb_d[s])
            z_sb = []
            for mt, rows in ((0, 128), (1, 72)):
                c0 = s * NN + mt * 128
                zp = zpool.tile([128, 3 * PSG], F32, tag="z", name=f"zp{s}_{mt}")
                for g in range(3):
                    for kc in range(3):
                        nc.tensor.matmul(
                            zp[0:rows, g * PSG:g * PSG + 361],
                            lhsT=xt_tiles[kc][:, c0:c0 + rows],
                            rhs=wt_tiles[kc][:, g * GP:g * GP + 361],
                            start=(kc == 0), stop=(kc == 2),
                        )
                zs = spool.tile([128, 5 * GP], BF16, tag=f"zs{mt}", name=f"zs{s}_{mt}")
                src = zp[0:rows, :].rearrange("p (g c) -> p g c", g=3)[:, :, 0:GP]
                dst = zs[0:rows, 0:3 * GP].rearrange("p (g c) -> p g c", g=3)
                nc.scalar.copy(dst, src)
                z_sb.append(zs)
            state[s] = (mf_t, mb_t, z_sb)

        def emit_agg(s):
            mf_t, mb_t, z_sb = state.pop(s)
            ot = tpool.tile([128, 2 * DOUT], BF16, tag="ot", name=f"ot{s}")
            for mt, rows in ((0, 128), (1, 72)):
                d0 = mt * 128
                ap_ = apool.tile([128, 2 * PSG], F32, tag="agg", name=f"ap{s}_{mt}")
                nc.tensor.matmul(ap_[0:rows, 0:361], lhsT=mf_t[0:128, d0:d0 + rows],
                                 rhs=z_sb[0][0:128, 0:361], start=True, stop=False)
                nc.tensor.matmul(ap_[0:rows, 0:361], lhsT=mf_t[0:72, NN + d0:NN + d0 + rows],
                                 rhs=z_sb[1][0:72, 0:361], start=False, stop=True)
                nc.tensor.matmul(ap_[0:rows, PSG:PSG + 361], lhsT=mb_t[0:128, d0:d0 + rows],
                                 rhs=z_sb[0][0:128, GP:GP + 361], start=True, stop=False)
                nc.tensor.matmul(ap_[0:rows, PSG:PSG + 361], lhsT=mb_t[0:72, NN + d0:NN + d0 + rows],
                                 rhs=z_sb[1][0:72, GP:GP + 361], start=False, stop=True)

                zs = z_sb[mt]
                src = ap_[0:rows, :].rearrange("p (g c) -> p g c", g=2)[:, :, 0:GP]
                dst = zs[0:rows, 3 * GP:5 * GP].rearrange("p (g c) -> p g c", g=2)
                if mt == 1 and s % 2 == 0:
                    nc.vector.tensor_copy(dst, src)
                else:
                    nc.scalar.copy(dst, src)

                # ---- gating epilogue ----
                # one sigmoid over the 3 gate cols (loop, agg-in, agg-out)
                sgt = tpool.tile([128, 3], F32, tag="sgt", name=f"sg{s}_{mt}")
                gates = zs[0:rows, :].rearrange("p (g c) -> p g c", c=GP)[:, 2:5, 360]
                nc.scalar.activation(sgt[0:rows], gates, AF.Sigmoid)
                sg_l, sg_i, sg_o = sgt[:, 0:1], sgt[:, 1:2], sgt[:, 2:3]

                t1 = tpool.tile([128, DOUT], BF16, tag="t1", name=f"t1{s}_{mt}")
                t2 = tpool.tile([128, DOUT], BF16, tag="t2", name=f"t2{s}_{mt}")
                t3 = tpool.tile([128, DOUT], BF16, tag="t3", name=f"t3{s}_{mt}")
                nc.vector.tensor_scalar_mul(
                    t1[0:rows], zs[0:rows, 4 * GP:4 * GP + 360], sg_o[0:rows])
                nc.vector.scalar_tensor_tensor(
                    out=t2[0:rows], in0=zs[0:rows, 3 * GP:3 * GP + 360], scalar=sg_i[0:rows],
                    in1=t1[0:rows], op0=OP.mult, op1=OP.add)
                nc.vector.scalar_tensor_tensor(
                    out=t3[0:rows], in0=zs[0:rows, 2 * GP:2 * GP + 360], scalar=sg_l[0:rows],
                    in1=t2[0:rows], op0=OP.mult, op1=OP.add)
                nc.vector.tensor_scalar_max(
                    ot[0:rows, mt * DOUT:(mt + 1) * DOUT], t3[0:rows], 0.0)
            nc.gpsimd.dma_start(out=out_d[s], in_=ot)

        # software pipeline: aggregation runs one sentence behind main
        for s in range(SPC):
            emit_main(s)
            if s > 0:
                emit_agg(s - 1)
        emit_agg(SPC - 1)

    nc.compile()
    return nc


def _get_compiled():
    global _compiled
    if _compiled is None:
        _compiled = _build()
    return _compiled


def kernel(gcn_in, adj_ind, adj_data, w_in, b_in, w_out, b_out, w_loop,
           w_gin, b_gin, w_gout, b_gout, w_gloop):
    from concourse.bass_utils import run_bass_kernel_spmd

    x = np.asarray(gcn_in, np.float32)           # [B, N, DIN]
    idx = np.asarray(adj_ind)[0]                 # [B, E, 2] int
    dat = np.asarray(adj_data, np.float32)[0]    # [B, E]

    # fused weight matrix with bias row
    wt = np.zeros((KA, 3 * GP), np.float32)
    for g, (w, gw, bias, gb) in enumerate([
        (w_in, w_gin, b_in, b_gin),
        (w_out, w_gout, b_out, b_gout),
        (w_loop, w_gloop, None, None),
    ]):
        wt[0:DIN, g * GP:g * GP + DOUT] = np.asarray(w, np.float32)
        wt[0:DIN, g * GP + DOUT] = np.asarray(gw, np.float32)[:, 0]
        if bias is not None:
            wt[DIN, g * GP:g * GP + DOUT] = np.asarray(bias, np.float32)[0]
            wt[DIN, g * GP + DOUT] = np.asarray(gb, np.float32)[0]
    wt = wt.astype(NBF16)

    # dense per-sentence adjacency matrices
    M = np.zeros((B, NN, NN), np.float32)
    bi = np.broadcast_to(np.arange(B)[:, None], idx.shape[:2])
    np.add.at(M, (bi, idx[:, :, 0].astype(np.int64), idx[:, :, 1].astype(np.int64)), dat)

    def chunked(mm):      # [SPC,200,200] -> [SPC,128,400]: two 128-row chunks side by side
        out = np.zeros((SPC, 128, 2 * NN), np.float32)
        out[:, :, 0:NN] = mm[:, 0:128, :]
        out[:, 0:72, NN:2 * NN] = mm[:, 128:200, :]
        return out.astype(NBF16)

    nc = _get_compiled()
    in_maps = []
    for c in range(NCORES):
        xc = x[c * SPC:(c + 1) * SPC].reshape(ROWS, DIN)
        xt = np.empty((KA, ROWS), np.float32)
        xt[0:DIN] = xc.T
        xt[DIN] = 1.0
        mc = M[c * SPC:(c + 1) * SPC]
        in_maps.append({
            "xt": np.ascontiguousarray(xt).astype(NBF16),
            "wt": wt,
            "mf": chunked(mc),
            "mb": chunked(np.ascontiguousarray(mc.transpose(0, 2, 1))),
        })

    res = run_bass_kernel_spmd(nc, in_maps, core_ids=list(range(NCORES)))
    kernel.last_results = res
    out = np.empty((B, NN, DOUT), np.float32)
    for c in range(NCORES):
        oc = res.results[c]["out"].astype(np.float32)   # [SPC,128,720]
        oc_s = out[c * SPC:(c + 1) * SPC]               # [SPC,200,360]
        oc_s[:, 0:128, :] = oc[:, :, 0:DOUT]
        oc_s[:, 128:200, :] = oc[:, 0:72, DOUT:2 * DOUT]
    return out


# revision 7
# speedup vs baseline: 1.2724x; 1.0188x over previous
"""Gated GCN layer (DDI message passing) on 8 Trainium2 NeuronCores.

Strategy (data-parallel over batch B=256 -> 32 sentences/core):
  - Host: per-sentence edge list -> dense [200,200] adjacency matrix M
    (M[src,dst] += data). Aggregations become dense matmuls:
      in_t  = M^T @ (X W_in + b)   ;  out_t = M @ (X W_out + b)
  - X^T is pre-transposed on host and augmented with a ones-row so the
    bias rides inside the matmul (K = 361).
  - One fused weight matrix wt [361, 3*364]: groups (W_in|w_gin),
    (W_out|w_gout), (W_loop|w_gloop) each 361 cols padded to 364; bias row.
  - Device per sentence: Z = Xaug^T.T @ wt (3 K-chunks accumulated in
    PSUM), aggregate with M/M^T as stationary operands (gate column rides
    along), then gating epilogue (one batched sigmoid + STT chain).
    Aggregation runs one sentence behind the main matmul so PSUM slot
    reuse never stalls the PE.
  - Everything bf16 on device (fp32 accumulation in PSUM).

SBUF z-tile layout per (sentence, row-tile): [128, 5*364] bf16 groups:
  g0 in-Z | g1 out-Z | g2 loop-Z | g3 agg-in | g4 agg-out
  each group: 360 feats + gate col at 360 (gates of g2..g4 -> one sigmoid).
"""
import sys

if "/opt/trn_rl_repo" not in sys.path:
    sys.path.insert(0, "/opt/trn_rl_repo")

from contextlib import ExitStack

import ml_dtypes
import numpy as np

B, NN, EE, DIN, DOUT = 256, 200, 400, 360, 360
NCORES = 8
SPC = B // NCORES          # 32 sentences per core
ROWS = SPC * NN            # 6400 rows per core
KA = DIN + 1               # 361: augmented contraction (ones row for bias)
GP = 364                   # group stride in SBUF (4B aligned for bf16)
PSG = 512                  # group stride in PSUM (fp32 bank)
KCH = [(0, 121), (121, 241), (241, 361)]   # K chunks <= 128
NBF16 = np.dtype(ml_dtypes.bfloat16)

_compiled = None


def _build():
    import concourse.bacc as bacc
    import concourse.mybir as mybir
    from concourse.tile import TileContext

    BF16 = mybir.dt.bfloat16
    F32 = mybir.dt.float32
    AF = mybir.ActivationFunctionType
    OP = mybir.AluOpType

    nc = bacc.Bacc(None, target_bir_lowering=False)
    xt_d = nc.dram_tensor("xt", [KA, ROWS], BF16, kind="ExternalInput")
    wt_d = nc.dram_tensor("wt", [KA, 3 * GP], BF16, kind="ExternalInput")
    mf_d = nc.dram_tensor("mf", [SPC, 128, 2 * NN], BF16, kind="ExternalInput")
    mb_d = nc.dram_tensor("mb", [SPC, 128, 2 * NN], BF16, kind="ExternalInput")
    out_d = nc.dram_tensor("out", [SPC, 128, 2 * DOUT], BF16, kind="ExternalOutput")

    with TileContext(nc) as tc, ExitStack() as ctx:
        cpool = ctx.enter_context(tc.tile_pool(name="cpool", bufs=1))
        mpool = ctx.enter_context(tc.tile_pool(name="mpool", bufs=3))
        spool = ctx.enter_context(tc.tile_pool(name="spool", bufs=4))
        tpool = ctx.enter_context(tc.tile_pool(name="tpool", bufs=4))
        zpool = ctx.enter_context(tc.tile_pool(name="zpool", bufs=2, space="PSUM"))
        apool = ctx.enter_context(tc.tile_pool(name="apool", bufs=1, space="PSUM"))

        # ---- resident inputs: weights first (tiny), then X^T with the
        # first sentences' columns landing first ----
        wt_tiles = []
        for kc, (k0, k1) in enumerate(KCH):
            t = cpool.tile([k1 - k0, 3 * GP], BF16, name=f"wt{kc}")
            nc.sync.dma_start(out=t, in_=wt_d[k0:k1, :])
            wt_tiles.append(t)
        xt_tiles = [cpool.tile([k1 - k0, ROWS], BF16, name=f"xt{kc}")
                    for kc, (k0, k1) in enumerate(KCH)]
        first_eng = [nc.scalar, nc.gpsimd, nc.sync]
        for kc, (k0, k1) in enumerate(KCH):
            first_eng[kc].dma_start(out=xt_tiles[kc][:, 0:400], in_=xt_d[k0:k1, 0:400])
        for j in list(range(400, 3200, 800)) + list(range(3200, ROWS, 1600)):
            w = 800 if j < 3200 else 1600
            for kc, (k0, k1) in enumerate(KCH):
                nc.sync.dma_start(out=xt_tiles[kc][:, j:j + w], in_=xt_d[k0:k1, j:j + w])

        state = {}   # per in-flight sentence: (mf_t, mb_t, [zs0, zs1])

        def emit_main(s):
            mf_t = mpool.tile([128, 2 * NN], BF16, tag="mf", name=f"mf{s}")
            mb_t = mpool.tile([128, 2 * NN], BF16, tag="mb", name=f"mb{s}")
            nc.gpsimd.dma_start(out=mf_t, in_=mf_d[s])
            nc.gpsimd.dma_start(out=mb_t, in_=mb_d[s])
            z_sb = []
            for mt, rows in ((0, 128), (1, 72)):
                c0 = s * NN + mt * 128
                zp = zpool.tile([128, 3 * PSG], F32, tag="z", name=f"zp{s}_{mt}")
                for kc in range(3):
                    for g in range(3):
                        nc.tensor.matmul(
                            zp[0:rows, g * PSG:g * PSG + 361],
                            lhsT=xt_tiles[kc][:, c0:c0 + rows],
                            rhs=wt_tiles[kc][:, g * GP:g * GP + 361],
                            start=(kc == 0), stop=(kc == 2),
                        )
                zs = spool.tile([128, 5 * GP], BF16, tag=f"zs{mt}", name=f"zs{s}_{mt}")
                src = zp[0:rows, :].rearrange("p (g c) -> p g c", g=3)[:, :, 0:GP]
                dst = zs[0:rows, 0:3 * GP].rearrange("p (g c) -> p g c", g=3)
                nc.scalar.copy(dst, src)
                z_sb.append(zs)
            state[s] = (mf_t, mb_t, z_sb)

        def emit_agg(s):
            mf_t, mb_t, z_sb = state.pop(s)
            ot = tpool.tile([128, 2 * DOUT], BF16, tag="ot", name=f"ot{s}")
            for mt, rows in ((0, 128), (1, 72)):
                d0 = mt * 128
                ap_ = apool.tile([128, 2 * PSG], F32, tag="agg", name=f"ap{s}_{mt}")
                nc.tensor.matmul(ap_[0:rows, 0:361], lhsT=mf_t[0:128, d0:d0 + rows],
                                 rhs=z_sb[0][0:128, 0:361], start=True, stop=False)
                nc.tensor.matmul(ap_[0:rows, 0:361], lhsT=mf_t[0:72, NN + d0:NN + d0 + rows],
                                 rhs=z_sb[1][0:72, 0:361], start=False, stop=True)
                nc.tensor.matmul(ap_[0:rows, PSG:PSG + 361], lhsT=mb_t[0:128, d0:d0 + rows],
                                 rhs=z_sb[0][0:128, GP:GP + 361], start=True, stop=False)
                nc.tensor.matmul(ap_[0:rows, PSG:PSG + 361], lhsT=mb_t[0:72, NN + d0:NN + d0 + rows],
                                 rhs=z_sb[1][0:72, GP:GP + 361], start=False, stop=True)

                zs = z_sb[mt]
                src = ap_[0:rows, :].rearrange("p (g c) -> p g c", g=2)[:, :, 0:GP]
                dst = zs[0:rows, 3 * GP:5 * GP].rearrange("p (g c) -> p g c", g=2)
                if mt == 1 and s % 2 == 0:
                    nc.vector.tensor_copy(dst, src)
                else:
                    nc.scalar.copy(dst, src)

                # ---- gating epilogue ----
                # one sigmoid over the 3 gate cols (loop, agg-in, agg-out)
                sgt = tpool.tile([128, 3], F32, tag="sgt", name=f"sg{s}_{mt}")
                gates = zs[0:rows, :].rearrange("p (g c) -> p g c", c=GP)[:, 2:5, 360]
                nc.scalar.activation(sgt[0:rows], gates, AF.Sigmoid)
                sg_l, sg_i, sg_o = sgt[:, 0:1], sgt[:, 1:2], sgt[:, 2:3]

                t1 = tpool.tile([128, DOUT], BF16, tag="t1", name=f"t1{s}_{mt}")
                t2 = tpool.tile([128, DOUT], BF16, tag="t2", name=f"t2{s}_{mt}")
                t3 = tpool.tile([128, DOUT], BF16, tag="t3", name=f"t3{s}_{mt}")
                nc.vector.tensor_scalar_mul(
                    t1[0:rows], zs[0:rows, 4 * GP:4 * GP + 360], sg_o[0:rows])
                nc.vector.tensor_scalar_mul(
                    t2[0:rows], zs[0:rows, 3 * GP:3 * GP + 360], sg_i[0:rows])
                nc.vector.tensor_scalar_mul(
                    t3[0:rows], zs[0:rows, 2 * GP:2 * GP + 360], sg_l[0:rows])
                t12 = tpool.tile([128, DOUT], BF16, tag="t12", name=f"t12{s}_{mt}")
                nc.vector.tensor_add(out=t12[0:rows], in0=t1[0:rows], in1=t2[0:rows])
                nc.vector.tensor_add(out=t3[0:rows], in0=t12[0:rows], in1=t3[0:rows])
                nc.vector.tensor_scalar_max(
                    ot[0:rows, mt * DOUT:(mt + 1) * DOUT], t3[0:rows], 0.0)
            nc.gpsimd.dma_start(out=out_d[s], in_=ot)

        # software pipeline: aggregation runs one sentence behind main
        for s in range(SPC):
            emit_main(s)
            if s > 0:
                emit_agg(s - 1)
        emit_agg(SPC - 1)

    nc.compile()
    return nc


def _get_compiled():
    global _compiled
    if _compiled is None:
        _compiled = _build()
    return _compiled


def kernel(gcn_in, adj_ind, adj_data, w_in, b_in, w_out, b_out, w_loop,
           w_gin, b_gin, w_gout, b_gout, w_gloop):
    from concourse.bass_utils import run_bass_kernel_spmd

    x = np.asarray(gcn_in, np.float32)           # [B, N, DIN]
    idx = np.asarray(adj_ind)[0]                 # [B, E, 2] int
    dat = np.asarray(adj_data, np.float32)[0]    # [B, E]

    # fused weight matrix with bias row
    wt = np.zeros((KA, 3 * GP), np.float32)
    for g, (w, gw, bias, gb) in enumerate([
        (w_in, w_gin, b_in, b_gin),
        (w_out, w_gout, b_out, b_gout),
        (w_loop, w_gloop, None, None),
    ]):
        wt[0:DIN, g * GP:g * GP + DOUT] = np.asarray(w, np.float32)
        wt[0:DIN, g * GP + DOUT] = np.asarray(gw, np.float32)[:, 0]
        if bias is not None:
            wt[DIN, g * GP:g * GP + DOUT] = np.asarray(bias, np.float32)[0]
            wt[DIN, g * GP + DOUT] = np.asarray(gb, np.float32)[0]
    wt = wt.astype(NBF16)

    # dense per-sentence adjacency matrices
    M = np.zeros((B, NN, NN), np.float32)
    bi = np.broadcast_to(np.arange(B)[:, None], idx.shape[:2])
    np.add.at(M, (bi, idx[:, :, 0].astype(np.int64), idx[:, :, 1].astype(np.int64)), dat)

    def chunked(mm):      # [SPC,200,200] -> [SPC,128,400]: two 128-row chunks side by side
        out = np.zeros((SPC, 128, 2 * NN), np.float32)
        out[:, :, 0:NN] = mm[:, 0:128, :]
        out[:, 0:72, NN:2 * NN] = mm[:, 128:200, :]
        return out.astype(NBF16)

    nc = _get_compiled()
    in_maps = []
    for c in range(NCORES):
        xc = x[c * SPC:(c + 1) * SPC].reshape(ROWS, DIN)
        xt = np.empty((KA, ROWS), np.float32)
        xt[0:DIN] = xc.T
        xt[DIN] = 1.0
        mc = M[c * SPC:(c + 1) * SPC]
        in_maps.append({
            "xt": np.ascontiguousarray(xt).astype(NBF16),
            "wt": wt,
            "mf": chunked(mc),
            "mb": chunked(np.ascontiguousarray(mc.transpose(0, 2, 1))),
        })

    res = run_bass_kernel_spmd(nc, in_maps, core_ids=list(range(NCORES)))
    kernel.last_results = res
    out = np.empty((B, NN, DOUT), np.float32)
    for c in range(NCORES):
        oc = res.results[c]["out"].astype(np.float32)   # [SPC,128,720]
        oc_s = out[c * SPC:(c + 1) * SPC]               # [SPC,200,360]
        oc_s[:, 0:128, :] = oc[:, :, 0:DOUT]
        oc_s[:, 128:200, :] = oc[:, 0:72, DOUT:2 * DOUT]
    return out
